# revision 49
# baseline (speedup 1.0000x reference)
"""GCN propagation kernel for Trainium2, 8 NeuronCores.

Computes out = D^-1/2 A D^-1/2 X W  with A [16384,16384] f32, X [16384,256] f32,
W [256,256] f32, D = diag(rowsum(A)).

Strategy (1D row partition, 8 cores):
  - Each core owns 2048 rows of A.  The host pre-transposes its shard so the
    contraction dim lands on SBUF partitions, in two precisions:
    adjT_c = A[rows_c, :].T as fp16 (for the big matmul) and the same shard as
    fp8e4m3 (degree pass only - a rowsum tolerates fp8 input error, ~1e-5 on
    deg, while halving that pass's HBM traffic).
  - Pass 1: stream the fp8 shard (32 MiB), rowsum on the PE with an all-ones
    stationary operand in fp8 DoubleRow mode (two k-rows per cell; k pairing
    is irrelevant when every weight is 1.0) -> local deg [2048].  feat (fp16,
    replicated) is prefetched into SBUF meanwhile.
  - AllGather local deg -> full deg [16384] (64 KiB collective).
  - d = rsqrt(max(deg, eps)) on chip (DVE reciprocal + ACT sqrt), then
    x1 = feat * d[row] in fp16, in place (per-partition scalars).
  - Pass 2: stream the fp16 shard (64 MiB); x2^T [256, 2048] accumulated in
    all 8 PSUM banks over the full 16384-deep contraction (x1 k-slices
    stationary, adjT moving, N=512 chunks).
  - Epilogue: x2T -> SBUF, GEMM2 with W in fp32, PE-transpose back to row
    orientation, scale by local d (the outer D^-1/2 commutes past W), DMA out.

All accumulation is fp32; reduced precision only touches the A-matmul inputs.
End-to-end error vs the fp32 reference: ~3.6e-4 (L2 relative).
Measured per-iteration device time (NEFF-internal loop slope): ~350-400 us;
the kernel is within ~15% of the 104 MiB / ~340 GB/s HBM roofline.
"""

import numpy as np

import concourse.bass as bass  # noqa: F401  (AP types come through tile/bacc)
import concourse.tile as tile
from concourse import bacc, mybir
from concourse.masks import make_identity

NCORES = 8
N = 16384          # nodes
D = 256            # feature dim (in == out)
M = N // NCORES    # 2048 local rows per core
P = 128            # partitions
KB = N // P        # 128 contraction blocks
MB = M // P        # 16 local row blocks
NCH = M // 512     # 4 moving-dim chunks of 512
EPS = 1e-12

F16 = mybir.dt.float16
F32 = mybir.dt.float32
F8 = mybir.dt.float8e4  # e4m3


def _build(loop_n=1, act_split=True, deg_fp8=True, dual_ring=False, deg_dr=True):
    """loop_n > 1 repeats the whole body inside the NEFF (timing use only)."""
    import contextlib

    nc = bacc.Bacc("TRN2", target_bir_lowering=False, debug=False, num_devices=NCORES)
    adjT = nc.dram_tensor("adjT", [N, M], F16, kind="ExternalInput").ap()
    adj8 = nc.dram_tensor("adj8", [N, M], F8, kind="ExternalInput").ap()
    featq = nc.dram_tensor("featq", [N, D], F16, kind="ExternalInput").ap()
    wgt = nc.dram_tensor("wgt", [D, D], F32, kind="ExternalInput").ap()
    out = nc.dram_tensor("out", [M, D], F32, kind="ExternalOutput").ap()

    with tile.TileContext(nc) as tc:
        with tc.tile_pool(name="const", bufs=1) as cpool, \
             tc.tile_pool(name="x1p", bufs=1) as x1p, \
             tc.tile_pool(name="adjp", bufs=12) as adjp, \
             tc.tile_pool(name="big", bufs=1) as big, \
             tc.tile_pool(name="small", bufs=1) as small, \
             tc.tile_pool(name="outp", bufs=6) as outp, \
             tc.tile_pool(name="ps", bufs=1, space="PSUM") as ps, \
             tc.tile_pool(name="dram", bufs=1, space="DRAM") as dram:

            ones = cpool.tile([P, 32], F8 if deg_fp8 else F16)
            nc.vector.memset(ones[:], 1.0)
            ident = cpool.tile([P, P], F32)
            make_identity(nc, ident[:])
            w_sb = cpool.tile([P, 2 * D], F32)  # w_sb[:, nb*D:(nb+1)*D] = W[nb*128:(nb+1)*128, :]
            nc.scalar.dma_start(
                w_sb[:].rearrange("p (nb o) -> p nb o", nb=2),
                wgt.rearrange("(nb p) o -> p nb o", p=P),
            )

            # x1 (= scaled feat) lives for the whole kernel: [128, KB*D] fp16
            x1_all = x1p.tile([P, KB * D], F16)

            # PSUM is managed as 8 shared bank slots (tags bank0..bank7);
            # tiles in different phases reuse banks as lifetimes allow.
            # deg accumulators: 4 PSUM banks of [1, 512]
            deg_full = [ps.tile([P, 512], F32, name=f"deg_ps{i}", tag=f"bank{i}") for i in range(NCH)]
            deg_ps = [t[:16, :] if (deg_fp8 and deg_dr) else t[:1, :] for t in deg_full]

            # ---------------- pass 1: rowsum(adjT) + feat prefetch ----------------
            deg_row = small.tile([1, M], F32)
            with (tc.For_i(0, loop_n, 1) if loop_n > 1 else contextlib.nullcontext()):
                if deg_fp8 and deg_dr:
                    # fp8 DoubleRow: two k-rows per PE cell, any k pairing is fine
                    # for a ones-weighted rowsum.
                    for kb2 in range(KB // 2):
                        a8t = adjp.tile([P, 2 * M], F8, name="a8t", tag="a8t", bufs=6)
                        nc.sync.dma_start(
                            a8t[:].rearrange("p (j m) -> p j m", j=2),
                            adj8[kb2 * 2 * P:(kb2 + 1) * 2 * P, :].rearrange("(j p) m -> p j m", p=P),
                        )
                        rhs3 = a8t[:].rearrange("p (j m) -> p j m", j=2)
                        lhs3 = ones[:].rearrange("p (j m) -> p j m", j=2)  # [128, 2, 16]
                        for mc in range(NCH):
                            nc.tensor.matmul(
                                deg_ps[mc][:, :],
                                lhs3,
                                rhs3[:, :, mc * 512:(mc + 1) * 512],
                                start=(kb2 == 0),
                                stop=(kb2 == KB // 2 - 1),
                                perf_mode=mybir.MatmulPerfMode.DoubleRow,
                            )
                        if kb2 % 8 == 0 and loop_n == 1:
                            c0 = kb2 * 2  # feat chunk: k-tiles [c0, c0+16)
                            nc.sync.dma_start(
                                x1_all[:, c0 * D:(c0 + 16) * D].rearrange("p (t f) -> p t f", t=16),
                                featq[c0 * P:(c0 + 16) * P, :].rearrange("(t p) f -> p t f", p=P),
                            )
                else:
                    for kb in range(KB):
                        dma_eng = nc.scalar if (dual_ring and kb % 2) else nc.sync
                        if deg_fp8:
                            a8t = adjp.tile([P, M], F8, name="a8t", tag="a8t")
                            dma_eng.dma_start(a8t[:], adj8[kb * P:(kb + 1) * P, :])
                        else:
                            a8t = adjp.tile([P, M], F16, name="a16t", tag="adjt")
                            dma_eng.dma_start(a8t[:], adjT[kb * P:(kb + 1) * P, :])
                        for mc in range(NCH):
                            nc.tensor.matmul(
                                deg_ps[mc][:, :],
                                ones[:, :1],
                                a8t[:, mc * 512:(mc + 1) * 512],
                                start=(kb == 0),
                                stop=(kb == KB - 1),
                            )
                    if kb % 16 == 0 and loop_n == 1:
                        c0 = kb  # feat chunk: k-tiles [c0, c0+16)
                        nc.sync.dma_start(
                            x1_all[:, c0 * D:(c0 + 16) * D].rearrange("p (t f) -> p t f", t=16),
                            featq[c0 * P:(c0 + 16) * P, :].rearrange("(t p) f -> p t f", p=P),
                        )

                # ---------------- deg -> AllGather -> d ----------------
                for mc in range(NCH):
                    nc.vector.tensor_copy(deg_row[:, mc * 512:(mc + 1) * 512], deg_full[mc][:1, :])
            deg_loc_dram = dram.tile([M], F32)
            nc.scalar.dma_start(deg_loc_dram.rearrange("(p f) -> p f", p=1), deg_row[:, :])
            deg_all_dram = dram.tile([N], F32, addr_space="Shared")
            nc.gpsimd.collective_compute(
                "AllGather",
                mybir.AluOpType.bypass,
                replica_groups=[list(range(NCORES))],
                ins=[deg_loc_dram[:]],
                outs=[deg_all_dram[:]],
            )
            # natural layout [p, t] = deg[p*128 + t]
            d_nat = small.tile([P, P], F32)
            nc.scalar.dma_start(d_nat[:], deg_all_dram.rearrange("(p t) -> p t", p=P))
            nc.vector.tensor_scalar_max(d_nat[:], d_nat[:], EPS)
            d_rec = small.tile([P, P], F32)
            nc.vector.reciprocal(d_rec[:], d_nat[:])
            d_rs = small.tile([P, P], F32)
            nc.scalar.sqrt(d_rs[:], d_rec[:])
            # transpose so that d_all[p, t] = rsqrt(deg[t*128 + p])
            d_ps = ps.tile([P, 512], F32, name="d_ps", tag="bank4")[:, :P]
            nc.tensor.transpose(d_ps[:], d_rs[:], ident[:])
            d_all = small.tile([P, P], F32)
            nc.vector.tensor_copy(d_all[:], d_ps[:])

            # local d for the epilogue: d_loc[p, mb] = rsqrt(deg_local[mb*128+p])
            degl = small.tile([P, MB], F32)
            nc.scalar.dma_start(degl[:], deg_loc_dram.rearrange("(t p) -> p t", p=P))
            nc.vector.tensor_scalar_max(degl[:], degl[:], EPS)
            degl_rec = small.tile([P, MB], F32)
            nc.vector.reciprocal(degl_rec[:], degl[:])
            d_loc = small.tile([P, MB], F32)
            nc.scalar.sqrt(d_loc[:], degl_rec[:])

            loop2 = tc.For_i(0, loop_n, 1) if loop_n > 1 else contextlib.nullcontext()
            loop2.__enter__()
            if loop_n > 1:  # feat load happens in-loop for the timing variant
                for c0 in range(0, KB, 16):
                    nc.sync.dma_start(
                        x1_all[:, c0 * D:(c0 + 16) * D].rearrange("p (t f) -> p t f", t=16),
                        featq[c0 * P:(c0 + 16) * P, :].rearrange("(t p) f -> p t f", p=P),
                    )

            # x1 = featq * d (in place, fp16)
            for t in range(KB):
                sl = x1_all[:, t * D:(t + 1) * D]
                nc.vector.tensor_scalar_mul(sl, sl, d_all[:, t:t + 1])

            # ---------------- pass 2: x2T[n, m] += x1[k, n]^T adjT[k, m] ----------------
            x2_ps = [[ps.tile([P, 512], F32, name=f"x2_ps{nb}_{mc}", tag=f"bank{nb * NCH + mc}")
                      for mc in range(NCH)] for nb in range(2)]
            for kb in range(KB):
                adjt = adjp.tile([P, M], F16, name="adjt", tag="adjt")
                dma_eng = nc.scalar if (dual_ring and kb % 2) else nc.sync
                dma_eng.dma_start(adjt[:], adjT[kb * P:(kb + 1) * P, :])
                for nb in range(2):
                    lhsT = x1_all[:, kb * D + nb * P: kb * D + (nb + 1) * P]
                    for mc in range(NCH):
                        nc.tensor.matmul(
                            x2_ps[nb][mc][:, :],
                            lhsT,
                            adjt[:, mc * 512:(mc + 1) * 512],
                            start=(kb == 0),
                            stop=(kb == KB - 1),
                        )

            # ---------------- epilogue ----------------
            x2_sb = [big.tile([P, M], F32, name=f"x2_sb{nb}", tag=f"x2_sb{nb}") for nb in range(2)]
            for nb in range(2):
                for mc in range(NCH):
                    # alternate engines so PSUM evacuation runs on ACT and DVE in parallel
                    eng = nc.scalar.copy if act_split and (nb * NCH + mc) % 2 else nc.vector.tensor_copy
                    eng(x2_sb[nb][:, mc * 512:(mc + 1) * 512], x2_ps[nb][mc][:, :])
            # GEMM2 (fp32): preT[o, m] = sum_n W[n, o] * x2T[n, m]
            pre_ps = [[ps.tile([P, 512], F32, name=f"pre_ps{ob}_{mc}", tag=f"bank{ob * NCH + mc}")
                       for mc in range(NCH)] for ob in range(2)]
            for ob in range(2):
                for mc in range(NCH):
                    for nb in range(2):
                        nc.tensor.matmul(
                            pre_ps[ob][mc][:, :],
                            w_sb[:, nb * D + ob * P: nb * D + (ob + 1) * P],
                            x2_sb[nb][:, mc * 512:(mc + 1) * 512],
                            start=(nb == 0),
                            stop=(nb == 1),
                        )
            pre_sb = [big.tile([P, M], F32, name=f"pre_sb{ob}", tag=f"pre_sb{ob}") for ob in range(2)]
            for ob in range(2):
                for mc in range(NCH):
                    eng = nc.scalar.copy if act_split and (ob * NCH + mc) % 2 else nc.vector.tensor_copy
                    eng(pre_sb[ob][:, mc * 512:(mc + 1) * 512], pre_ps[ob][mc][:, :])
            # transpose back to [m, o] blocks, scale by local d, store
            for mb in range(MB):
                out_t = outp.tile([P, D], F32, name="out_t", tag="out_t")
                for ob in range(2):
                    tp = ps.tile([P, 512], F32, name="tp", tag=f"bank{(mb * 2 + ob) % 8}")[:, :P]
                    nc.tensor.transpose(tp[:], pre_sb[ob][:, mb * P:(mb + 1) * P], ident[:])
                    if not (act_split and (mb * 2 + ob) % 2):
                        nc.vector.tensor_scalar_mul(out_t[:, ob * P:(ob + 1) * P], tp[:], d_loc[:, mb:mb + 1])
                    else:
                        nc.scalar.mul(out_t[:, ob * P:(ob + 1) * P], tp[:], d_loc[:, mb:mb + 1])
                nc.sync.dma_start(out[mb * P:(mb + 1) * P, :], out_t[:])
            loop2.__exit__(None, None, None)

    nc.compile()
    return nc


_NC_CACHE = []


def _get_nc():
    if not _NC_CACHE:
        _NC_CACHE.append(_build())
    return _NC_CACHE[0]


def _make_in_maps(adj, feat, weight):
    import ml_dtypes
    from concurrent.futures import ThreadPoolExecutor

    featq = feat.astype(np.float16)
    adj16 = adj.astype(np.float16)

    def prep(c):
        adjT_c = np.ascontiguousarray(adj16[c * M:(c + 1) * M, :].T)
        adj8_c = adjT_c.astype(ml_dtypes.float8_e4m3)
        return {"adjT": adjT_c, "adj8": adj8_c, "featq": featq, "wgt": weight}

    with ThreadPoolExecutor(max_workers=8) as ex:
        return list(ex.map(prep, range(NCORES)))


_RUNNER_CACHE = []


def _get_runner():
    """Jitted SPMD executable for the compiled bass module (built once).

    Mirrors concourse.bass2jax.run_bass_via_pjrt's multi-core path, but keeps
    the jitted function cached so repeated kernel() calls skip recompilation.
    """
    if _RUNNER_CACHE:
        return _RUNNER_CACHE[0]

    import jax
    from jax.experimental.shard_map import shard_map
    from jax.sharding import Mesh, NamedSharding, PartitionSpec
    from concourse.bass2jax import install_neuronx_cc_hook, _bass_exec_p, partition_id_tensor

    nc = _get_nc()
    install_neuronx_cc_hook()
    assert nc.dbg_addr is None
    partition_name = nc.partition_id_tensor.name if nc.partition_id_tensor else None

    in_names, out_names, out_avals, out_shapes = [], [], [], []
    for alloc in nc.m.functions[0].allocations:
        if not isinstance(alloc, mybir.MemoryLocationSet):
            continue
        name = alloc.memorylocations[0].name
        if alloc.kind == "ExternalInput":
            if name == partition_name:
                continue
            in_names.append(name)
        elif alloc.kind == "ExternalOutput":
            out_names.append(name)
            shape = tuple(alloc.tensor_shape)
            dtype = mybir.dt.np(alloc.dtype)
            out_avals.append(jax.core.ShapedArray(shape, dtype))
            out_shapes.append((shape, dtype))
    all_names = in_names + out_names
    if partition_name is not None:
        all_names = all_names + [partition_name]

    def _body(*args):
        operands = list(args)
        if partition_name is not None:
            operands.append(partition_id_tensor())
        outs = _bass_exec_p.bind(
            *operands,
            out_avals=tuple(out_avals),
            in_names=tuple(all_names),
            out_names=tuple(out_names),
            lowering_input_output_aliases=(),
            sim_require_finite=True,
            sim_require_nnan=True,
            nc=nc,
        )
        return tuple(outs)

    devices = jax.devices()[:NCORES]
    mesh = Mesh(np.asarray(devices), ("core",))
    n_args = len(in_names) + len(out_names)
    fn = jax.jit(
        shard_map(_body, mesh=mesh,
                  in_specs=(PartitionSpec("core"),) * n_args,
                  out_specs=(PartitionSpec("core"),) * len(out_names),
                  check_rep=False),
        keep_unused=True,
    )
    sharding = NamedSharding(mesh, PartitionSpec("core"))
    runner = (fn, sharding, in_names, out_names, out_shapes)
    _RUNNER_CACHE.append(runner)
    return runner


def kernel(adj, feat, weight):
    import jax

    adj = np.asarray(adj, dtype=np.float32)
    feat = np.asarray(feat, dtype=np.float32)
    weight = np.asarray(weight, dtype=np.float32)
    assert adj.shape == (N, N) and feat.shape == (N, D) and weight.shape == (D, D)

    in_maps = _make_in_maps(adj, feat, weight)
    fn, sharding, in_names, out_names, out_shapes = _get_runner()

    dev_args = []
    for name in in_names:
        cat = np.concatenate([m[name] for m in in_maps], axis=0)
        dev_args.append(jax.device_put(cat, sharding))
    for shape, dtype in out_shapes:
        z = np.zeros((NCORES * shape[0], *shape[1:]), dtype)
        dev_args.append(jax.device_put(z, sharding))

    outs = fn(*dev_args)
    full = np.asarray(outs[out_names.index("out")])  # [NCORES*M, D] in core order
    return full
